# revision 1
# baseline (speedup 1.0000x reference)
"""Trainium2 Bass kernel for nn_ComposeImgLoss (8-core data-parallel).

Contract: kernel(**inputs) takes the FULL inputs
    GT   (8, 4, 128, 128) f32
    Pred (8, 6, 14, 4, 128, 128) f32
and returns the FULL scalar loss (f32), matching reference.reference().

Strategy (1 sample per core, 8 cores):
  phase 1 (overlapped with the 21MB/core DMA load):
    - per-attri min/max partials via fused tensor_tensor_reduce (DVE)
    - gen_L+gen_A+gen_W raw sum via identity-matmul accumulation (PE)
    - GT_norm, sigmoid ACT-table warmup
  collective 1: AllReduce(max) of [max, -min] (2 floats)
  phase 2:
    - threshold row computed on partition 0, broadcast via ones-matmul (PE)
    - color-count sums via tensor_scalar/scalar_tensor_tensor with fused
      accum_out per (slot, chan) block; partition-reduce via ones-matmul
    - type_list vals/membership on a [1,42] row (sum/q arithmetic trick)
    - sigmoid on ACT (runtime scale/bias APs), region product + weighted
      slot-sum on DVE
  collective 2: AllReduce(max) of region-sum [max, -min]
    - composite = clip(region_contrib + G3, 0, 1); SSE with fused
      square+accum; partition-sum via ones-matmul -> per-core scalar
  host: loss = sum(sse_core) / (8*3*128*128)
"""

import numpy as np

import concourse.bass as bass
import concourse.bacc as bacc
import concourse.tile as tile
from concourse import mybir
from concourse.masks import make_identity
from concourse.bass_utils import run_bass_kernel_spmd

import os
NO_COLL = os.environ.get("NO_COLL", "0") == "1"
SKIP_TTR = os.environ.get("SKIP_TTR", "0") == "1"
SKIP_LAW = os.environ.get("SKIP_LAW", "0") == "1"
SKIP_COUNTS = os.environ.get("SKIP_COUNTS", "0") == "1"
SKIP_REGION = os.environ.get("SKIP_REGION", "0") == "1"

F32 = mybir.dt.float32
BF16 = mybir.dt.bfloat16
OP = mybir.AluOpType
AF = mybir.ActivationFunctionType
AX = mybir.AxisListType

N_CORES = 8
SQE, H, W = 14, 128, 128
SC = 4 * W            # cols per slot (4 channels x 128 w)
AT = SQE * SC         # 7168 cols per attri
RES = [0, 2, 3, 4, 5]  # SBUF-resident attris; attri 1 is streamed
RB = {a: i * AT for i, a in enumerate(RES)}
NPIX = N_CORES * 3 * H * W  # denominator of the global mean


def build():
    nc = bacc.Bacc("TRN2", target_bir_lowering=False, debug=False,
                   num_devices=N_CORES)
    pred = nc.dram_tensor("Pred", [6, SQE, 4, H, W], F32, kind="ExternalInput")
    gt = nc.dram_tensor("GT", [4, H, W], F32, kind="ExternalInput")
    sse = nc.dram_tensor("sse", [1, 1], F32, kind="ExternalOutput")
    dbg = nc.dram_tensor("dbg", [1, 16], F32, kind="ExternalOutput")
    dbg2 = nc.dram_tensor("dbg2", [1, 256], F32, kind="ExternalOutput")

    with tile.TileContext(nc) as tc:
        with (
            tc.tile_pool(name="big", bufs=1) as big,
            tc.tile_pool(name="a1s", bufs=2) as a1s,
            tc.tile_pool(name="mid", bufs=1) as mid,
            tc.tile_pool(name="small", bufs=1) as small,
            tc.tile_pool(name="psum", bufs=1, space="PSUM") as psum,
            tc.tile_pool(name="dram", bufs=1, space="DRAM") as dram,
        ):
            # ---------------- tiles ----------------
            r5 = big.tile([128, 5 * AT], F32)          # 140KB/part
            ttr_s = mid.tile([128, 1792], F32)         # TTR elementwise dump
            sig = mid.tile([128, SQE * 3 * W], BF16)   # sigmoid out (s,c,w)
            alm = mid.tile([128, SQE * W], F32)        # alpha mask (s,w)
            reg = mid.tile([128, SQE * W], BF16)       # region_soft (s,w)
            law = mid.tile([128, 3 * W], F32)          # gen_L+A+W raw sum -> composite
            g3 = mid.tile([128, 3 * W], F32)
            acc = mid.tile([128, 3 * W], F32)          # weighted slot-sum
            gtn = mid.tile([128, 3 * W], F32)

            ident = small.tile([128, 128], F32)
            ones_c = small.tile([128, 1], F32)
            ones_r = small.tile([1, 128], F32)
            mm_parts = small.tile([128, 54], F32)       # minmax partials
            mm2 = small.tile([128, 2], F32)
            gmm = small.tile([2, 1], F32)
            rmm2 = small.tile([128, 2], F32)
            rgmm = small.tile([2, 1], F32)
            mtmp = small.tile([128, 1], F32)
            cnt = small.tile([128, 210], F32)
            msc1 = small.tile([128, 128], F32)
            msc2 = small.tile([128, 128], F32)
            gsb = small.tile([1, 2], F32)
            rsb = small.tile([1, 2], F32)
            rowb = small.tile([1, 16], F32)
            rtmp = small.tile([1, 1], F32)
            dd = small.tile([1, 1], F32)
            invd = small.tile([1, 1], F32)
            rinvd = small.tile([1, 1], F32)
            rrow = small.tile([1, 2], F32)
            crow = small.tile([1, 210], F32)
            rows = small.tile([1, 300], F32)
            srow = small.tile([1, 112], F32)
            thrb = small.tile([128, 9], F32)
            typb = small.tile([128, 42], F32)
            rcp = small.tile([128, 2], F32)
            ssecol = small.tile([128, 1], F32)
            sse_sb = small.tile([1, 1], F32)
            warm = small.tile([128, 1], F32)

            cin1 = dram.tile([16, 1], F32)
            cout1 = dram.tile([16, 1], F32, addr_space="Shared")
            cin2 = dram.tile([16, 1], F32)
            cout2 = dram.tile([16, 1], F32, addr_space="Shared")
            gpad = small.tile([16, 1], F32)
            rpad = small.tile([16, 1], F32)

            # ---------------- constants ----------------
            make_identity(nc, ident[:])
            nc.vector.memset(ones_c[:], 1.0)
            nc.vector.memset(ones_r[:], 1.0)
            # ACT sigmoid table warmup (overlaps DMA)
            nc.scalar.activation(warm[:], ones_c[:], AF.Sigmoid)

            def A(a):  # resident attri region [128, AT]
                return r5[:, RB[a]:RB[a] + AT]

            def blk(a, s, c):  # one (slot, chan) block [128, 128]
                off = RB[a] + s * SC + c * W
                return r5[:, off:off + W]

            # ---------------- phase 1: load + minmax + LAW ----------------
            pr = pred.ap()
            for a in RES:
                nc.sync.dma_start(
                    out=A(a).rearrange("h (s c w) -> h s c w", s=SQE, c=4),
                    in_=pr[a].rearrange("s c h w -> h s c w"))

            # min/max partials via tensor_scalar accum reductions (2x fp32)
            NMM = 27  # 5 attris x 4 chunks + 7 a1 chunks
            if SKIP_TTR:
                nc.vector.memset(mm_parts[:, 0:NMM], 1.0)
                nc.vector.memset(mm_parts[:, NMM:2 * NMM], 0.0)
            mmcol = 0
            for i, a in enumerate(RES):
                if SKIP_TTR:
                    break
                for q in range(4):
                    i0 = A(a)[:, q * 1792:(q + 1) * 1792]
                    nc.vector.tensor_scalar(
                        out=ttr_s[:], in0=i0, scalar1=1.0, scalar2=None,
                        op0=OP.mult, op1=OP.max,
                        accum_out=mm_parts[:, mmcol:mmcol + 1])
                    nc.vector.tensor_scalar(
                        out=ttr_s[:], in0=i0, scalar1=1.0, scalar2=None,
                        op0=OP.mult, op1=OP.min,
                        accum_out=mm_parts[:, NMM + mmcol:NMM + mmcol + 1])
                    mmcol += 1

            # attri 1: streamed in 7 chunks of 2 slots, only feeds min/max
            pa1 = pr[1].rearrange("s c h w -> h s c w")
            for k in range(7 if not SKIP_TTR else 0):
                ch = a1s.tile([128, 1024], F32, tag="a1chunk")
                nc.sync.dma_start(
                    out=ch[:].rearrange("h (s c w) -> h s c w", s=2, c=4),
                    in_=pa1[:, 2 * k:2 * k + 2])
                nc.vector.tensor_scalar(
                    out=ttr_s[:, 0:1024], in0=ch[:], scalar1=1.0, scalar2=None,
                    op0=OP.mult, op1=OP.max,
                    accum_out=mm_parts[:, mmcol:mmcol + 1])
                nc.vector.tensor_scalar(
                    out=ttr_s[:, 0:1024], in0=ch[:], scalar1=1.0, scalar2=None,
                    op0=OP.mult, op1=OP.min,
                    accum_out=mm_parts[:, NMM + mmcol:NMM + mmcol + 1])
                mmcol += 1

            # raw gen_L + gen_A + gen_W sum on PE (42 identity matmuls)
            if SKIP_LAW:
                nc.vector.memset(law[:], 21.0)
            else:
                p_law = psum.tile([128, 3 * W], F32)
                mms = [(a, s) for a in (2, 3, 5) for s in range(SQE)]
                for j, (a, s) in enumerate(mms):
                    rhs = r5[:, RB[a] + s * SC:RB[a] + s * SC + 3 * W]
                    nc.tensor.matmul(p_law[:], ident[:], rhs,
                                     start=(j == 0), stop=(j == len(mms) - 1))
                nc.vector.tensor_copy(law[:], p_law[:])

            # GT_norm = (GT[:3] + 1) / 2
            nc.sync.dma_start(out=gtn[:].rearrange("h (c w) -> h c w", w=W),
                              in_=gt.ap()[0:3].rearrange("c h w -> h c w"))
            nc.vector.tensor_scalar(out=gtn[:], in0=gtn[:], scalar1=0.5,
                                    scalar2=0.5, op0=OP.mult, op1=OP.add)

            # fold minmax partials: mm2 = [rowmax, -rowmin]
            nc.vector.tensor_reduce(out=mm2[:, 0:1], in_=mm_parts[:, 0:27],
                                    axis=AX.X, op=OP.max)
            nc.vector.tensor_reduce(out=mtmp[:], in_=mm_parts[:, 27:54],
                                    axis=AX.X, op=OP.min)
            nc.vector.tensor_scalar(out=mm2[:, 1:2], in0=mtmp[:], scalar1=-1.0,
                                    scalar2=None, op0=OP.mult)
            p_t = psum.tile([2, 128], F32)
            nc.tensor.transpose(p_t[:], mm2[:], ident[:])
            nc.vector.memset(gpad[:], -3.0e38)
            nc.vector.tensor_reduce(out=gpad[0:2, 0:1], in_=p_t[:], axis=AX.X,
                                    op=OP.max)

            # ---------------- collective 1 ----------------
            nc.gpsimd.dma_start(out=cin1[:], in_=gpad[:])
            if NO_COLL:
                nc.gpsimd.dma_start(
                    out=gsb[:], in_=cin1[0:2, 0:1].rearrange("p o -> o p"))
            else:
                nc.gpsimd.collective_compute(
                    "AllReduce", OP.max, replica_groups=[list(range(N_CORES))],
                    ins=[cin1.opt()], outs=[cout1.opt()])
                nc.gpsimd.dma_start(
                    out=gsb[:], in_=cout1[0:2, 0:1].rearrange("p o -> o p"))

            # ---------------- threshold row ----------------
            gmax, ngmn = gsb[:, 0:1], gsb[:, 1:2]
            nc.vector.tensor_tensor(out=dd[:], in0=gmax, in1=ngmn, op=OP.add)
            nc.vector.reciprocal(invd[:], dd[:])
            # rowb: 0:mn 1:t02 2:t04 3:t06 4:t08 5:sgscale 6:sgbias 7:invd 8:g3bias
            nc.vector.tensor_scalar(out=rowb[:, 0:1], in0=ngmn, scalar1=-1.0,
                                    scalar2=None, op0=OP.mult)
            for ck, col in ((0.2, 1), (0.4, 2), (0.6, 3), (0.8, 4)):
                nc.vector.tensor_scalar(out=rowb[:, col:col + 1], in0=dd[:],
                                        scalar1=ck, scalar2=rowb[:, 0:1],
                                        op0=OP.mult, op1=OP.add)
            nc.vector.tensor_scalar(out=rowb[:, 5:6], in0=invd[:], scalar1=10.0,
                                    scalar2=None, op0=OP.mult)
            nc.vector.tensor_tensor(out=rtmp[:], in0=ngmn, in1=rowb[:, 5:6],
                                    op=OP.mult)
            nc.vector.tensor_scalar(out=rowb[:, 6:7], in0=rtmp[:], scalar1=-9.0,
                                    scalar2=None, op0=OP.add)
            nc.vector.tensor_copy(rowb[:, 7:8], invd[:])
            nc.vector.tensor_tensor(out=rtmp[:], in0=ngmn, in1=invd[:],
                                    op=OP.mult)
            nc.vector.tensor_scalar(out=rowb[:, 8:9], in0=rtmp[:], scalar1=42.0,
                                    scalar2=None, op0=OP.mult)
            p_b = psum.tile([128, 9], F32)
            nc.tensor.matmul(p_b[:], ones_r[:], rowb[:, 0:9],
                             start=True, stop=True)
            nc.vector.tensor_copy(thrb[:], p_b[:])

            # G3 = (law - 42*mn) / d   (ready before collective 2)
            nc.vector.tensor_scalar(out=g3[:], in0=law[:],
                                    scalar1=thrb[:, 7:8], scalar2=thrb[:, 8:9],
                                    op0=OP.mult, op1=OP.add)

            # ---------------- alpha mask + count sums ----------------
            a0a = r5[:, RB[0]:RB[0] + AT].rearrange(
                "h (s c w) -> h s c w", s=SQE, c=4)[:, :, 3, :]  # [128,14,128]
            nc.vector.tensor_scalar(
                out=alm[:].rearrange("h (s w) -> h s w", w=W), in0=a0a,
                scalar1=thrb[:, 4:5], scalar2=None, op0=OP.is_gt)

            def ccol(m, s, c):
                return m * 42 + s * 3 + c

            if SKIP_COUNTS:
                nc.vector.memset(cnt[:], 0.0)
            for s in range(SQE if not SKIP_COUNTS else 0):
                for c in range(3):
                    b = blk(0, s, c)
                    for m, tcol in ((0, 2), (1, 3), (2, 4)):  # t04, t06, t08
                        nc.vector.tensor_scalar(
                            out=msc1[:], in0=b, scalar1=thrb[:, tcol:tcol + 1],
                            scalar2=None, op0=OP.is_gt, op1=OP.add,
                            accum_out=cnt[:, ccol(m, s, c):ccol(m, s, c) + 1])
                    am = alm[:, s * W:(s + 1) * W]
                    for m, tcol in ((3, 0), (4, 1)):  # mn, t02 (alpha-gated)
                        nc.vector.scalar_tensor_tensor(
                            out=msc2[:], in0=b, scalar=thrb[:, tcol:tcol + 1],
                            in1=am, op0=OP.is_gt, op1=OP.mult,
                            accum_out=cnt[:, ccol(m, s, c):ccol(m, s, c) + 1])

            p_cnt = psum.tile([1, 210], F32)
            nc.tensor.matmul(p_cnt[:], ones_c[:], cnt[:], start=True, stop=True)
            nc.vector.tensor_copy(crow[:], p_cnt[:])

            # ---------------- type_list on the [1,42] row ----------------
            S1, S2, S3 = crow[:, 0:42], crow[:, 42:84], crow[:, 84:126]
            S4, S5 = crow[:, 126:168], crow[:, 168:210]
            c1, c0 = rows[:, 0:42], rows[:, 42:84]
            t1, t2, b2 = rows[:, 84:126], rows[:, 126:168], rows[:, 168:210]
            nb2, vals = rows[:, 210:252], rows[:, 252:294]
            nc.vector.tensor_tensor(out=c1, in0=S1, in1=S2, op=OP.subtract)
            nc.vector.tensor_tensor(out=c0, in0=S4, in1=S5, op=OP.subtract)
            nc.vector.tensor_tensor(out=t1, in0=S3, in1=c1, op=OP.is_gt)
            nc.vector.tensor_tensor(out=t2, in0=S3, in1=c0, op=OP.is_gt)
            nc.vector.tensor_tensor(out=b2, in0=t1, in1=t2, op=OP.mult)
            nc.vector.tensor_scalar(out=nb2, in0=b2, scalar1=-1.0, scalar2=1.0,
                                    op0=OP.mult, op1=OP.add)
            nc.vector.tensor_tensor(out=t1, in0=c1, in1=c0, op=OP.is_gt)
            nc.vector.tensor_tensor(out=t2, in0=nb2, in1=t1, op=OP.mult)  # b1
            nc.vector.scalar_tensor_tensor(out=vals, in0=t2, scalar=0.5,
                                           in1=b2, op0=OP.mult, op1=OP.add)
            vv = vals.rearrange("h (s c) -> h s c", c=3)
            v0, v1, v2 = vv[:, :, 0], vv[:, :, 1], vv[:, :, 2]
            sv, s6 = srow[:, 0:14], srow[:, 14:28]
            qq, q2 = srow[:, 28:42], srow[:, 42:56]
            e3, band = srow[:, 56:70], srow[:, 70:84]
            etmp, mem = srow[:, 84:98], srow[:, 98:112]
            nc.vector.tensor_tensor(out=sv, in0=v0, in1=v1, op=OP.add)
            nc.vector.tensor_tensor(out=sv, in0=sv, in1=v2, op=OP.add)
            nc.vector.tensor_scalar(out=s6, in0=sv, scalar1=2.0, scalar2=None,
                                    op0=OP.mult)
            nc.vector.scalar_tensor_tensor(out=qq, in0=v0, scalar=2.0, in1=v1,
                                           op0=OP.mult, op1=OP.add)
            nc.vector.scalar_tensor_tensor(out=q2, in0=qq, scalar=2.0, in1=v2,
                                           op0=OP.mult, op1=OP.add)
            nc.vector.tensor_scalar(out=qq, in0=q2, scalar1=2.0, scalar2=None,
                                    op0=OP.mult)
            nc.vector.tensor_scalar(out=mem, in0=s6, scalar1=0.0, scalar2=None,
                                    op0=OP.is_equal)
            for sval in (4.0, 6.0):
                nc.vector.tensor_scalar(out=etmp, in0=s6, scalar1=sval,
                                        scalar2=None, op0=OP.is_equal)
                nc.vector.tensor_tensor(out=mem, in0=mem, in1=etmp, op=OP.add)
            nc.vector.tensor_scalar(out=e3, in0=s6, scalar1=3.0, scalar2=None,
                                    op0=OP.is_equal)
            nc.vector.tensor_scalar(out=band, in0=qq, scalar1=7.0, scalar2=None,
                                    op0=OP.is_ge)
            nc.vector.tensor_scalar(out=etmp, in0=qq, scalar1=9.0, scalar2=None,
                                    op0=OP.is_le)
            nc.vector.tensor_tensor(out=band, in0=band, in1=etmp, op=OP.mult)
            nc.vector.tensor_tensor(out=e3, in0=e3, in1=band, op=OP.mult)
            nc.vector.tensor_tensor(out=mem, in0=mem, in1=e3, op=OP.add)
            # type = vals * member (broadcast member over c)
            tyrow = rows[:, 84:126]  # reuse
            nc.vector.tensor_tensor(
                out=tyrow.rearrange("h (s c) -> h s c", c=3), in0=vv,
                in1=mem[:, :, None].to_broadcast([1, 14, 3]), op=OP.mult)
            p_ty = psum.tile([128, 42], F32)
            nc.tensor.matmul(p_ty[:], ones_r[:], tyrow, start=True, stop=True)
            nc.vector.tensor_copy(typb[:], p_ty[:])

            # ---------------- region_soft + weighted sum ----------------
            a4r = r5[:, RB[4]:RB[4] + AT].rearrange(
                "h (s c w) -> h s c w", s=SQE, c=4)[:, :, 0:3, :]
            sig4 = sig[:].rearrange("h (s c w) -> h s c w", s=SQE, c=3)
            if SKIP_REGION:
                nc.vector.memset(acc[:], 0.5)
            else:
                nc.scalar.activation(sig4, a4r, AF.Sigmoid,
                                     bias=thrb[:, 6:7], scale=thrb[:, 5:6])
                reg3 = reg[:].rearrange("h (s w) -> h s w", w=W)
                nc.vector.tensor_tensor(out=reg3, in0=sig4[:, :, 0, :],
                                        in1=sig4[:, :, 1, :], op=OP.mult)
                nc.vector.tensor_tensor(out=reg3, in0=reg3,
                                        in1=sig4[:, :, 2, :], op=OP.mult)
            for c in range(3 if not SKIP_REGION else 0):
                a_c = acc[:, c * W:(c + 1) * W]
                nc.vector.tensor_scalar(
                    out=a_c, in0=reg[:, 0:W], scalar1=typb[:, c:c + 1],
                    scalar2=None, op0=OP.mult)
                for s in range(1, SQE):
                    nc.vector.scalar_tensor_tensor(
                        out=a_c, in0=reg[:, s * W:(s + 1) * W],
                        scalar=typb[:, s * 3 + c:s * 3 + c + 1], in1=a_c,
                        op0=OP.mult, op1=OP.add)

            # ---------------- collective 2 (region min/max) ----------------
            nc.vector.tensor_reduce(out=rmm2[:, 0:1], in_=acc[:], axis=AX.X,
                                    op=OP.max)
            nc.vector.tensor_reduce(out=mtmp[:], in_=acc[:], axis=AX.X,
                                    op=OP.min)
            nc.vector.tensor_scalar(out=rmm2[:, 1:2], in0=mtmp[:], scalar1=-1.0,
                                    scalar2=None, op0=OP.mult)
            p_t2 = psum.tile([2, 128], F32)
            nc.tensor.transpose(p_t2[:], rmm2[:], ident[:])
            nc.vector.memset(rpad[:], -3.0e38)
            nc.vector.tensor_reduce(out=rpad[0:2, 0:1], in_=p_t2[:], axis=AX.X,
                                    op=OP.max)
            nc.gpsimd.dma_start(out=cin2[:], in_=rpad[:])
            if NO_COLL:
                nc.gpsimd.dma_start(
                    out=rsb[:], in_=cin2[0:2, 0:1].rearrange("p o -> o p"))
            else:
                nc.gpsimd.collective_compute(
                    "AllReduce", OP.max, replica_groups=[list(range(N_CORES))],
                    ins=[cin2.opt()], outs=[cout2.opt()])
                nc.gpsimd.dma_start(
                    out=rsb[:], in_=cout2[0:2, 0:1].rearrange("p o -> o p"))

            nc.vector.tensor_tensor(out=dd[:], in0=rsb[:, 0:1], in1=rsb[:, 1:2],
                                    op=OP.add)
            nc.vector.reciprocal(rinvd[:], dd[:])
            nc.vector.tensor_copy(rrow[:, 0:1], rinvd[:])
            nc.vector.tensor_tensor(out=rrow[:, 1:2], in0=rsb[:, 1:2],
                                    in1=rinvd[:], op=OP.mult)
            p_b2 = psum.tile([128, 2], F32)
            nc.tensor.matmul(p_b2[:], ones_r[:], rrow[:], start=True, stop=True)
            nc.vector.tensor_copy(rcp[:], p_b2[:])

            # ---------------- composite + SSE ----------------
            nc.vector.tensor_scalar(out=g3[:], in0=g3[:], scalar1=rcp[:, 1:2],
                                    scalar2=None, op0=OP.add)
            nc.vector.scalar_tensor_tensor(out=law[:], in0=acc[:],
                                           scalar=rcp[:, 0:1], in1=g3[:],
                                           op0=OP.mult, op1=OP.add)
            nc.vector.tensor_scalar(out=law[:], in0=law[:], scalar1=0.0,
                                    scalar2=1.0, op0=OP.max, op1=OP.min)
            nc.vector.tensor_tensor(out=law[:], in0=law[:], in1=gtn[:],
                                    op=OP.subtract)
            nc.vector.scalar_tensor_tensor(out=g3[:], in0=law[:], scalar=1.0,
                                           in1=law[:], op0=OP.mult,
                                           op1=OP.mult, accum_out=ssecol[:])
            p_s = psum.tile([1, 1], F32)
            nc.tensor.matmul(p_s[:], ones_c[:], ssecol[:], start=True, stop=True)
            nc.vector.tensor_copy(sse_sb[:], p_s[:])
            nc.sync.dma_start(out=sse.ap(), in_=sse_sb[:])

            # ---------------- debug outputs ----------------
            nc.sync.dma_start(out=dbg.ap()[:, 0:9], in_=rowb[:, 0:9])
            nc.sync.dma_start(out=dbg.ap()[:, 9:11], in_=rsb[:])
            nc.sync.dma_start(out=dbg.ap()[:, 11:13], in_=gsb[:])
            nc.sync.dma_start(out=dbg2.ap()[:, 0:210], in_=crow[:])
            nc.sync.dma_start(out=dbg2.ap()[:, 210:252], in_=tyrow)

    nc.finalize()
    return nc


_NC = None


def _get_nc():
    global _NC
    if _NC is None:
        _NC = build()
    return _NC


def run(gt_full, pred_full, trace=False):
    """Run the SPMD kernel on the full (8, ...) inputs. Returns
    (loss, BassKernelResults)."""
    nc = _get_nc()
    in_maps = [
        {"GT": np.ascontiguousarray(gt_full[i]),
         "Pred": np.ascontiguousarray(pred_full[i])}
        for i in range(N_CORES)
    ]
    res = run_bass_kernel_spmd(nc, in_maps, core_ids=list(range(N_CORES)),
                               trace=trace)
    total = sum(float(res.results[c]["sse"][0, 0]) for c in range(N_CORES))
    loss = np.float32(total / NPIX)
    return loss, res


def kernel(GT, Pred):
    gt_full = np.asarray(GT, dtype=np.float32)
    pred_full = np.asarray(Pred, dtype=np.float32)
    loss, _ = run(gt_full, pred_full, trace=False)
    return loss


if __name__ == "__main__":
    rng = np.random.default_rng(0)
    gt = rng.random((8, 4, H, W), dtype=np.float32)
    pr = rng.random((8, 6, SQE, 4, H, W), dtype=np.float32)
    print("loss:", kernel(gt, pr))

